# revision 20
# baseline (speedup 1.0000x reference)
"""Trainium2 Bass kernel for nn_CIFARClassifier (8-block dense CNN, C=3).

Sharding: pure data parallel — batch 4096 split as 512 images per core
across 8 NeuronCores; the tiny weights/BN params are replicated (folded
host-side into per-block conv matrices + bias vectors).

Per-core layout: activations live in SBUF as [(c,h) partitions, (b,w) free]
with w padded by one zero column on each side (SAME-conv padding).  The h
index is placed with its low bits as the HIGH partition bits
(r32(c,h) = (h&1)*64 + ((h>>1)&1)*32 + c*8 + (h>>2)), so each 2x2 maxpool is
a free-dim max over w-pairs plus a partition-half max over h-pairs.  The
h-half realignment is done on the PE (identity matmul into PSUM) because
DMA instructions here only support a single sem-wait and DVE operands must
share a start partition.  A 3x3 conv = 3 PE matmuls (one per kernel column
kx, PSUM-accumulated, rhs shifted by kx into the padded columns); the
stationary operand is a host-built KxM matrix encoding (ic,ky)->(oc,ho)
mixing for all h rows at once.  BN folds into the matrix (scale) and an
ACT-fused relu(x+bias) (shift).  GAP(1/64) + the final 1x1 conv fold into
one [24,10] matmul whose lhsT is the data — logits land directly as
[batch, 10] for the log-softmax tail.

DMA discipline (walrus limit: one sem-wait per DMA): every input-load DMA
is dependency-free — x1 uses one virgin buffer per sub-tile (bufs=4), pool
hole rows are filled by duplicate data DMAs (their conv-matrix rows are
zero, contents only need to be finite), pad columns by byte-disjoint
memsets.  The single output DMA at the end carries the one allowed wait.
"""

import numpy as np

EPS = 1e-5
B_TOTAL = 4096
N_CORES = 8
B_CORE = B_TOTAL // N_CORES  # 512
NB = 128                     # batch sub-tile per inner iteration
N_SUB = B_CORE // NB         # 4
P32, P16, P8 = 120, 56, 24   # used partitions (with pool-alignment holes)


def _rmap32(c, h):
    return (h & 1) * 64 + ((h >> 1) & 1) * 32 + c * 8 + (h >> 2)


def _rmap16(c, h):
    return (h & 1) * 32 + c * 8 + (h >> 1)


def _rmap8(c, h):
    return c * 8 + h


def _conv_mats(wp, rmap, R, P):
    """wp: [oc=3, ic=3, ky=3, kx=3] BN-folded weights -> [kx, K=P, M=P]."""
    mats = np.zeros((3, P, P), np.float32)
    for oc in range(3):
        for ho in range(R):
            m = rmap(oc, ho)
            for ic in range(3):
                for ky in range(3):
                    hi = ho + ky - 1
                    if 0 <= hi < R:
                        k = rmap(ic, hi)
                        mats[:, k, m] = wp[oc, ic, ky, :]
    return mats


def _build_consts(ws, w9, gammas, betas, means, variances):
    ws = np.asarray(ws, np.float64)
    w9 = np.asarray(w9, np.float64)
    cm32 = np.zeros((2, 3, P32, P32), np.float32)
    cm16 = np.zeros((3, 3, P16, P16), np.float32)
    cm8 = np.zeros((3, 3, P8, P8), np.float32)
    bias32 = np.zeros((2, P32), np.float32)
    bias16 = np.zeros((3, P16), np.float32)
    bias8 = np.zeros((3, P8), np.float32)
    for blk in range(8):
        inv = np.asarray(gammas[blk], np.float64) / np.sqrt(
            np.asarray(variances[blk], np.float64) + EPS
        )
        wp = ws[blk] * inv[:, None, None, None]
        bb = np.asarray(betas[blk], np.float64) - np.asarray(means[blk], np.float64) * inv
        if blk < 2:
            cm32[blk] = _conv_mats(wp, _rmap32, 32, P32)
            for oc in range(3):
                for h in range(32):
                    bias32[blk, _rmap32(oc, h)] = bb[oc]
        elif blk < 5:
            cm16[blk - 2] = _conv_mats(wp, _rmap16, 16, P16)
            for oc in range(3):
                for h in range(16):
                    bias16[blk - 2, _rmap16(oc, h)] = bb[oc]
        else:
            cm8[blk - 5] = _conv_mats(wp, _rmap8, 8, P8)
            for oc in range(3):
                for h in range(8):
                    bias8[blk - 5, _rmap8(oc, h)] = bb[oc]
    ghead = np.zeros((P8, 10), np.float32)
    for c in range(3):
        for h in range(8):
            ghead[_rmap8(c, h), :] = w9[:, c, 1, 1] / 64.0
    # identities for the PE-side h-pool realignment, pre-placed at the
    # partition base of the half they copy (lhsT/rhs share start partition)
    id56 = np.zeros((P32, P16), np.float32)
    id56[64:120] = np.eye(P16, dtype=np.float32)
    id24 = np.zeros((P16, P8), np.float32)
    id24[32:56] = np.eye(P8, dtype=np.float32)
    return {
        "cm32": cm32, "cm16": cm16, "cm8": cm8,
        "bias32": bias32, "bias16": bias16, "bias8": bias8,
        "ghead": ghead, "id56": id56, "id24": id24,
    }


def build_program():
    import concourse.bass as bass
    import concourse.tile as tile
    from concourse import mybir

    f32 = mybir.dt.float32
    AFT = mybir.ActivationFunctionType
    ALU = mybir.AluOpType
    AX = mybir.AxisListType

    nc = bass.Bass()
    x_d = nc.dram_tensor("x", [N_SUB, 128, NB, 34], f32, kind="ExternalInput")
    cm32_d = nc.dram_tensor("cm32", [2, 3, P32, P32], f32, kind="ExternalInput")
    cm16_d = nc.dram_tensor("cm16", [3, 3, P16, P16], f32, kind="ExternalInput")
    cm8_d = nc.dram_tensor("cm8", [3, 3, P8, P8], f32, kind="ExternalInput")
    b32_d = nc.dram_tensor("bias32", [2, P32], f32, kind="ExternalInput")
    b16_d = nc.dram_tensor("bias16", [3, P16], f32, kind="ExternalInput")
    b8_d = nc.dram_tensor("bias8", [3, P8], f32, kind="ExternalInput")
    gh_d = nc.dram_tensor("ghead", [P8, 10], f32, kind="ExternalInput")
    id56_d = nc.dram_tensor("id56", [P32, P16], f32, kind="ExternalInput")
    id24_d = nc.dram_tensor("id24", [P16, P8], f32, kind="ExternalInput")
    out_d = nc.dram_tensor("out", [B_CORE, 10], f32, kind="ExternalOutput")

    with tile.TileContext(nc) as tc:
        with (
            tc.tile_pool(name="consts", bufs=1) as cpool,
            tc.tile_pool(name="acts", bufs=1) as apool,
            tc.tile_pool(name="xin", bufs=N_SUB) as xpool,
            tc.tile_pool(name="ps", bufs=4, space="PSUM") as pspool,
            tc.tile_pool(name="ptp", bufs=2, space="PSUM") as ptpool,
            tc.tile_pool(name="ph", bufs=2, space="PSUM") as phpool,
            tc.tile_pool(name="small", bufs=2) as spool,
            tc.tile_pool(name="resp", bufs=1) as rpool,
        ):
            # ---- constants (gpsimd/SWDGE: dep-free, keeps HWDGE lanes
            # virgin for the one dep-carrying output DMA) ----
            cm_t = {}
            bias_t = {}
            grp_of = lambda blk: 0 if blk < 2 else (1 if blk < 5 else 2)
            cm_d = (cm32_d, cm16_d, cm8_d)
            bv_d = (b32_d, b16_d, b8_d)
            base_blk = (0, 2, 5)
            Ps = (P32, P16, P8)
            for blk in range(8):
                g = grp_of(blk)
                P = Ps[g]
                bi = blk - base_blk[g]
                for kx in range(3):
                    t = cpool.tile([P, P], f32, tag=f"cm{blk}_{kx}")
                    nc.gpsimd.dma_start(out=t[:, :], in_=cm_d[g][bi, kx, :, :])
                    cm_t[(blk, kx)] = t
                bt = cpool.tile([P, 1], f32, tag=f"bias{blk}")
                nc.gpsimd.dma_start(out=bt[:, :], in_=bv_d[g][bi, :])
                bias_t[blk] = bt
            gh_t = cpool.tile([P8, 10], f32, tag="ghead")
            nc.gpsimd.dma_start(out=gh_t[:, :], in_=gh_d[:, :])
            id56_t = cpool.tile([P32, P16], f32, tag="id56")
            nc.gpsimd.dma_start(out=id56_t[:, :], in_=id56_d[:, :])
            id24_t = cpool.tile([P16, P8], f32, tag="id24")
            nc.gpsimd.dma_start(out=id24_t[:, :], in_=id24_d[:, :])

            res_all = rpool.tile([128, N_SUB, 10], f32, tag="res_all")

            def conv_block(blk, P, R, nbc, src, dst):
                """dst[0:P, :, 1:R+1] = relu(conv(src) + bias); also zeroes
                dst's pad columns so dst can feed the next conv/sum."""
                nc.any.memset(dst[:, :, 0:1], 0.0)
                nc.any.memset(dst[:, :, R + 1:R + 2], 0.0)
                for j in range(NB // nbc):
                    b0, b1 = j * nbc, (j + 1) * nbc
                    pt = pspool.tile([P, nbc * R], f32, tag="pt")
                    for kx in range(3):
                        nc.tensor.matmul(
                            pt[:, :], cm_t[(blk, kx)][:, :],
                            src[0:P, b0:b1, kx:kx + R],
                            start=(kx == 0), stop=(kx == 2))
                    nc.scalar.activation(
                        out=dst[0:P, b0:b1, 1:R + 1],
                        in_=pt[:, :].rearrange("p (b w) -> p b w", w=R),
                        func=AFT.Relu, bias=bias_t[blk][:, :], scale=1.0)

            for t_i in range(N_SUB):
                b_off = t_i * NB

                # ---- load x sub-tile (host pre-permuted to the exact
                # SBUF layout, holes and pad columns pre-zeroed): one
                # contiguous dependency-free DMA ----
                x1 = xpool.tile([128, NB, 34], f32, tag="x1")
                nc.gpsimd.dma_start(out=x1[:, :, :], in_=x_d[t_i, :, :, :])

                # ---- 32x32 stage ----
                x2 = apool.tile([128, NB, 34], f32, tag="A")
                conv_block(0, P32, 32, 16, x1, x2)
                s12 = apool.tile([128, NB, 34], f32, tag="B")
                nc.vector.tensor_add(s12[0:P32], x1[0:P32], x2[0:P32])
                x3 = apool.tile([128, NB, 34], f32, tag="C")
                conv_block(1, P32, 32, 16, s12, x3)
                s123 = apool.tile([128, NB, 34], f32, tag="A")
                nc.vector.tensor_add(s123[0:P32], s12[0:P32], x3[0:P32])
                # maxpool 32->16: w-pairs on DVE, h-pairs via PE realign
                wp = apool.tile([128, NB, 16], f32, tag="C")
                s123v = s123[0:P32, :, 1:33].rearrange("p b (x two) -> p b x two", two=2)
                nc.vector.tensor_max(wp[0:P32, :, :], s123v[:, :, :, 0], s123v[:, :, :, 1])
                x4 = apool.tile([128, NB, 18], f32, tag="B")
                nc.any.memset(x4[:, :, 0:1], 0.0)
                nc.any.memset(x4[:, :, 17:18], 0.0)
                for j in range(4):
                    b0, b1 = j * 32, (j + 1) * 32
                    ptp = ptpool.tile([P16, 512], f32, tag="ptp")
                    nc.tensor.matmul(ptp[:, :], id56_t[64:120, :],
                                     wp[64:120, b0:b1, :], start=True, stop=True)
                    nc.vector.tensor_max(
                        x4[0:P16, b0:b1, 1:17], wp[0:P16, b0:b1, :],
                        ptp[:, :].rearrange("p (b w) -> p b w", w=16))

                # ---- 16x16 stage ----
                x5 = apool.tile([128, NB, 18], f32, tag="A")
                conv_block(2, P16, 16, 32, x4, x5)
                s45 = apool.tile([128, NB, 18], f32, tag="E")
                nc.vector.tensor_add(s45[0:P16], x4[0:P16], x5[0:P16])
                x6 = apool.tile([128, NB, 18], f32, tag="C")
                conv_block(3, P16, 16, 32, s45, x6)
                t56 = apool.tile([128, NB, 18], f32, tag="F")
                nc.vector.tensor_add(t56[0:P16], x5[0:P16], x6[0:P16])
                s456 = apool.tile([128, NB, 18], f32, tag="A")
                nc.vector.tensor_add(s456[0:P16], s45[0:P16], x6[0:P16])
                x7 = apool.tile([128, NB, 18], f32, tag="D")
                conv_block(4, P16, 16, 32, s456, x7)
                s567 = apool.tile([128, NB, 18], f32, tag="B")
                nc.vector.tensor_add(s567[0:P16], t56[0:P16], x7[0:P16])
                wp2 = apool.tile([128, NB, 8], f32, tag="A")
                s567v = s567[0:P16, :, 1:17].rearrange("p b (x two) -> p b x two", two=2)
                nc.vector.tensor_max(wp2[0:P16, :, :], s567v[:, :, :, 0], s567v[:, :, :, 1])
                x8 = apool.tile([128, NB, 10], f32, tag="D")
                nc.any.memset(x8[:, :, 0:1], 0.0)
                nc.any.memset(x8[:, :, 9:10], 0.0)
                for j in range(2):
                    b0, b1 = j * 64, (j + 1) * 64
                    ptp = ptpool.tile([P8, 512], f32, tag="ptp")
                    nc.tensor.matmul(ptp[:, :], id24_t[32:56, :],
                                     wp2[32:56, b0:b1, :], start=True, stop=True)
                    nc.vector.tensor_max(
                        x8[0:P8, b0:b1, 1:9], wp2[0:P8, b0:b1, :],
                        ptp[:, :].rearrange("p (b w) -> p b w", w=8))

                # ---- 8x8 stage ----
                x9 = apool.tile([128, NB, 10], f32, tag="C")
                conv_block(5, P8, 8, 64, x8, x9)
                s89 = apool.tile([128, NB, 10], f32, tag="F")
                nc.vector.tensor_add(s89[0:P8], x8[0:P8], x9[0:P8])
                x10 = apool.tile([128, NB, 10], f32, tag="E")
                conv_block(6, P8, 8, 64, s89, x10)
                s8910 = apool.tile([128, NB, 10], f32, tag="C")
                nc.vector.tensor_add(s8910[0:P8], s89[0:P8], x10[0:P8])
                x11 = apool.tile([128, NB, 10], f32, tag="D")
                conv_block(7, P8, 8, 64, s8910, x11)

                # ---- GAP + head + log_softmax ----
                gsum = spool.tile([P8, NB], f32, tag="g")
                nc.vector.reduce_sum(out=gsum[:, :], in_=x11[0:P8, :, 1:9], axis=AX.X)
                ph = phpool.tile([128, 10], f32, tag="ph")
                nc.tensor.matmul(ph[:, :], gsum[:, :], gh_t[:, :], start=True, stop=True)
                mx = spool.tile([128, 1], f32, tag="m")
                nc.vector.reduce_max(out=mx[:, :], in_=ph[:, :], axis=AX.X)
                negm = spool.tile([128, 1], f32, tag="negm")
                nc.vector.tensor_scalar_mul(negm[:, :], mx[:, :], -1.0)
                ex = spool.tile([128, 10], f32, tag="e")
                ssum = spool.tile([128, 1], f32, tag="ssum")
                nc.scalar.activation(
                    out=ex[:, :], in_=ph[:, :], func=AFT.Exp,
                    bias=negm[:, :], scale=1.0, accum_out=ssum[:, :])
                ls = spool.tile([128, 1], f32, tag="ls")
                nc.scalar.activation(out=ls[:, :], in_=ssum[:, :], func=AFT.Ln)
                nc.vector.tensor_scalar(
                    out=res_all[:, t_i, :], in0=ph[:, :], scalar1=negm[:, :],
                    scalar2=ls[:, :], op0=ALU.add, op1=ALU.subtract)

            # single output DMA (the one wait it carries is the DVE tick of
            # the last res_all write; HWDGE lane 0 is virgin)
            dst = bass.AP(tensor=out_d, offset=0,
                          ap=[[10, 128], [NB * 10, N_SUB], [1, 10]])
            nc.sync.dma_start(out=dst, in_=res_all[:, :, :])

    return nc


def _prep_x(shard):
    """[B_CORE,3,32,32] -> [N_SUB,128,NB,34] in the kernel's SBUF layout
    (h-permuted partitions, zero pool-hole rows, zero w-pad columns)."""
    xs = shard.reshape(N_SUB, NB, 3, 32, 32)
    xp = np.zeros((N_SUB, 128, NB, 34), np.float32)
    for c in range(3):
        for h in range(32):
            xp[:, _rmap32(c, h), :, 1:33] = xs[:, :, c, h, :]
    return np.ascontiguousarray(xp)


def _make_in_maps(x, consts):
    x = np.ascontiguousarray(np.asarray(x, np.float32))
    in_maps = []
    for i in range(N_CORES):
        shard = x[i * B_CORE:(i + 1) * B_CORE]
        m = {"x": _prep_x(shard)}
        m.update(consts)
        in_maps.append(m)
    return in_maps


_PATCHED = False


def _split_multiwait(bir_json):
    """Walrus in this container accepts at most ONE sem-wait per
    instruction (setupSyncWait: 'Too many sync wait commands').  Tile's
    scheduler freely emits several.  Split the extras into single-wait
    EventSemaphore instructions on the same engine, immediately before the
    original instruction — same queue, so the sequencer performs the waits
    in order before issuing it."""
    import json
    d = json.loads(bir_json)
    cnt = 0
    for fn in d.get("functions", []):
        bkey = "basic_blocks" if "basic_blocks" in fn else "blocks"
        for blk in fn.get(bkey, []):
            out = []
            for inst in blk["instructions"]:
                si = inst.get("sync_info")
                ws = (si or {}).get("on_wait") or []
                if len(ws) > 1:
                    for w in ws[:-1]:
                        cnt += 1
                        out.append({
                            "debug": inst.get("debug", 0),
                            "engine": inst["engine"],
                            "ins": [], "outs": [],
                            "name": f"swsplit_{cnt}",
                            "opcode": "EventSemaphore",
                            "sync_info": {"on_wait": [w], "on_update": []},
                        })
                    si["on_wait"] = [ws[-1]]
                out.append(inst)
            blk["instructions"] = out
    return json.dumps(d).encode()


def _install_compile_patch():
    global _PATCHED
    if _PATCHED:
        return
    import concourse.bass_utils as _bu
    import concourse.bass2jax as _b2j

    orig = _bu.compile_bir_kernel

    def patched(bir_json, tmpdir, neff_name="file.neff"):
        return orig(_split_multiwait(bir_json), tmpdir, neff_name)

    _bu.compile_bir_kernel = patched
    _b2j.compile_bir_kernel = patched
    _PATCHED = True


def run(x, consts, trace=False):
    from concourse.bass_utils import run_bass_kernel_spmd

    _install_compile_patch()
    nc = build_program()
    res = run_bass_kernel_spmd(
        nc, _make_in_maps(x, consts), list(range(N_CORES)), trace=trace)
    out = np.concatenate([res.results[i]["out"] for i in range(N_CORES)], axis=0)
    return out, res


def time_warm(x, consts, iters=10):
    """Time warm executions of the compiled NEFF across all 8 cores.

    Rebuilds the pjrt callable (NEFF comes from the compile cache), keeps
    inputs resident on device, and times repeated dispatches."""
    import time
    import jax
    from jax.sharding import Mesh, PartitionSpec, NamedSharding
    from jax.experimental.shard_map import shard_map
    from concourse import bass2jax, mybir

    _install_compile_patch()
    nc = build_program()
    bass2jax.install_neuronx_cc_hook()
    in_maps = _make_in_maps(x, consts)

    partition_name = (nc.partition_id_tensor.name
                      if nc.partition_id_tensor else None)
    in_names, out_names, out_avals, zero_outs = [], [], [], []
    for alloc in nc.m.functions[0].allocations:
        if not isinstance(alloc, mybir.MemoryLocationSet):
            continue
        name = alloc.memorylocations[0].name
        if alloc.kind == "ExternalInput":
            if name != partition_name:
                in_names.append(name)
        elif alloc.kind == "ExternalOutput":
            shape = tuple(alloc.tensor_shape)
            dtype = mybir.dt.np(alloc.dtype)
            out_names.append(name)
            out_avals.append(jax.core.ShapedArray(shape, dtype))
            zero_outs.append(np.zeros(shape, dtype))
    n_params = len(in_names)
    n_outs = len(out_names)
    all_names = in_names + out_names
    if partition_name is not None:
        all_names = all_names + [partition_name]
    donate = tuple(range(n_params, n_params + n_outs))

    def _body(*args):
        operands = list(args)
        if partition_name is not None:
            operands.append(bass2jax.partition_id_tensor())
        outs = bass2jax._bass_exec_p.bind(
            *operands,
            out_avals=tuple(out_avals),
            in_names=tuple(all_names),
            out_names=tuple(out_names),
            lowering_input_output_aliases=(),
            sim_require_finite=True,
            sim_require_nnan=True,
            nc=nc,
        )
        return tuple(outs)

    devices = jax.devices()[:N_CORES]
    mesh = Mesh(np.asarray(devices), ("core",))
    in_specs = (PartitionSpec("core"),) * (n_params + n_outs)
    out_specs = (PartitionSpec("core"),) * n_outs
    sharded = jax.jit(
        shard_map(_body, mesh=mesh, in_specs=in_specs, out_specs=out_specs,
                  check_rep=False),
        donate_argnums=donate, keep_unused=True)

    sh = NamedSharding(mesh, PartitionSpec("core"))
    concat_in = [
        jax.device_put(
            np.concatenate([np.asarray(in_maps[c][name]) for c in range(N_CORES)],
                           axis=0), sh)
        for name in in_names
    ]
    for a in concat_in:
        a.block_until_ready()

    def zeros():
        return [np.zeros((N_CORES * z.shape[0], *z.shape[1:]), z.dtype)
                for z in zero_outs]

    r = sharded(*concat_in, *zeros())  # warmup (compile-cache hit)
    jax.block_until_ready(r)
    best = float("inf")
    for _ in range(iters):
        zs = zeros()
        t0 = time.perf_counter()
        r = sharded(*concat_in, *zs)
        jax.block_until_ready(r)
        best = min(best, time.perf_counter() - t0)
    return best * 1e9


def kernel(x, ws, w9, gammas, betas, means, variances):
    consts = _build_consts(ws, w9, gammas, betas, means, variances)
    out, _ = run(x, consts, trace=False)
    return np.asarray(out, np.float32)
